# revision 12
# baseline (speedup 1.0000x reference)
"""Trainium2 Bass kernel: single-head causal attention (v5).

Problem: x[B=8,T=2048,C=1024] @ Wq/Wk/Wv[C,H=64] -> causal softmax attention
-> out[B,T,H].  Sharding: pure data-parallel over B, one batch element per
NeuronCore (8 cores, no collectives).

v5 design (engine-balanced, software-pipelined):
  - host feeds x[b].T (C on partitions); [Wq*scale|Wk] concatenated.
  - qk projection: 8 accumulating f32r MMs per 512-q block -> psum
    (q on parts 0:64, k on parts 64:128); DVE distributes q to both
    partition halves and k-tiles to even/odd halves (for row packing).
  - v projection: 8 f32r MMs -> psum -> DVE copy to vT (bf16); v brought to
    natural [k,h] layout by DMA-xbar transposes (bf16, idle DMA engines),
    with a ones column appended -> att@v yields denominators for free.
  - scores in sT layout [k,q], row-packed pairs (even k-tile on array rows
    0:64, odd on 64:128), causal-trimmed; exp on ACT (psum->sbuf, bf16 out);
    diagonal-corner masking (128x128 lower-tri mul) on GPSIMD.
  - att@v (bf16): [v|1] stationary, accumulating [65,512] per block; row 64
    = softmax denominators.
  - software pipeline: av(b-1) matmuls are interleaved between block b's
    score pairs so PE never stalls on DVE copies or ACT exp; the pipeline
    carries across rep boundaries.
  - outT [65,512] DMA'd per block; divide by denominator + transpose happen
    on host.
"""

import numpy as np

P = 128
B = 8
T = 2048
C = 1024
H = 64
QB = 512          # q-block width
NB = T // QB      # 4 q-blocks
CC = C // P       # 8 contraction chunks
KT = T // P       # 16 key tiles
N_CORES = 8

_CACHE = {}


def _build(reps=1):
    import concourse.bacc as bacc
    import concourse.mybir as mybir
    import concourse.tile as tile

    dt = mybir.dt
    f32 = dt.float32
    f32r = dt.float32r
    bf16 = dt.bfloat16
    AF = mybir.ActivationFunctionType
    ALU = mybir.AluOpType

    nc = bacc.Bacc(None, target_bir_lowering=False)
    xT_d = nc.dram_tensor("xT", [C, T], f32r, kind="ExternalInput")
    wqk_d = nc.dram_tensor("wqk", [C, 2 * H], f32r, kind="ExternalInput")
    wv_d = nc.dram_tensor("wv", [C, H], f32r, kind="ExternalInput")
    outT_d = nc.dram_tensor("outT", [H + 1, T], f32, kind="ExternalOutput")

    with tile.TileContext(nc) as tc:
        with (
            tc.tile_pool(name="consts", bufs=1) as consts,
            tc.tile_pool(name="xpool", bufs=1) as xpool,
            tc.tile_pool(name="qkvp", bufs=1) as qkvp,
            tc.tile_pool(name="qdp", bufs=2) as qdp,
            tc.tile_pool(name="expp", bufs=12) as expp,
            tc.tile_pool(name="otp", bufs=2) as otp,
            tc.tile_pool(name="psA", bufs=2, space="PSUM") as psA,
            tc.tile_pool(name="psS", bufs=2, space="PSUM") as psS,
            tc.tile_pool(name="psO", bufs=2, space="PSUM") as psO,
        ):
            ident = consts.tile([P, P], bf16)
            from concourse.masks import make_identity
            make_identity(nc, ident)
            # tri[p, c] = 1.0 if c >= p else 0.0  (lower-tri in q>=k sense)
            tri = consts.tile([P, P], bf16)
            nc.gpsimd.memset(tri, 1.0)
            nc.gpsimd.affine_select(
                out=tri,
                in_=tri,
                compare_op=ALU.is_ge,
                fill=0.0,
                base=0,
                pattern=[[1, P]],
                channel_multiplier=-1,
            )

            wqk_sb = consts.tile([P, CC, 2 * H], f32r)
            nc.sync.dma_start(wqk_sb[:], wqk_d[:, :].rearrange("(c p) h -> p c h", p=P))
            wv_sb = consts.tile([P, CC, H], f32r)
            nc.sync.dma_start(wv_sb[:], wv_d[:, :].rearrange("(c p) h -> p c h", p=P))

            x_sb = xpool.tile([P, CC, T], f32r)
            for bb in range(NB):
                for c in range(CC):
                    nc.sync.dma_start(
                        x_sb[:, c, bb * QB:(bb + 1) * QB],
                        xT_d[c * P:(c + 1) * P, bb * QB:(bb + 1) * QB],
                    )

            # kT2[0:64, j, :] = k^T of even tile 2j ; [64:128, j, :] odd 2j+1
            kT2 = qkvp.tile([P, KT // 2, P], f32r)
            # v natural [k, h] + ones column, per key tile (bf16)
            v_sb = qkvp.tile([P, KT, H + 1], bf16)
            ones_col = consts.tile([P, KT, 1], bf16)
            nc.gpsimd.memset(ones_col[:], 1.0)
            nc.vector.tensor_copy(v_sb[:, :, H:H + 1], ones_col[:])
            vT = qkvp.tile([H, T], bf16)

            def project_qk(b):
                # [Wq|Wk] stationary -> q on psum parts 0:64, k on 64:128
                bsl = slice(b * QB, (b + 1) * QB)
                ps = psA.tile([P, QB], f32, tag="a", name="ps_qk")
                for c in range(CC):
                    nc.tensor.matmul(
                        ps, wqk_sb[:, c, :], x_sb[:, c, bsl],
                        start=(c == 0), stop=(c == CC - 1),
                    )
                qd = qdp.tile([P, QB], f32r, name="qd")
                nc.vector.tensor_copy(qd[0:H, :], ps[0:H, :])
                nc.vector.tensor_copy(qd[H:P, :], ps[0:H, :])
                # k split: even tiles -> parts 0:64, odd tiles -> parts 64:128
                j0 = 2 * b
                ksrc = ps[H:P, :].rearrange("p (j two c) -> p j two c", j=2, two=2, c=P)
                nc.vector.tensor_copy(kT2[0:H, j0:j0 + 2, :], ksrc[:, :, 0, :])
                nc.vector.tensor_copy(kT2[H:P, j0:j0 + 2, :], ksrc[:, :, 1, :])
                return qd

            def project_v(b):
                bsl = slice(b * QB, (b + 1) * QB)
                ps = psA.tile([P, QB], f32, tag="a", name="ps_v")
                for c in range(CC):
                    nc.tensor.matmul(
                        ps[0:H, :], wv_sb[:, c, :], x_sb[:, c, bsl],
                        start=(c == 0), stop=(c == CC - 1),
                    )
                nc.vector.tensor_copy(vT[:, bsl], ps[0:H, :])

            def v_to_natural(b):
                # PE transposes (bf16, 1 cyc/row); placed after the score/av
                # interleave so psA buffers are already free -> no PE stall
                for s in range(4):
                    t = b * 4 + s
                    pv = psA.tile([P, H], bf16, tag="a", name="ps_vt")
                    nc.tensor.matmul(
                        pv, vT[:, t * P:(t + 1) * P], ident[:H, :H],
                        is_transpose=True,
                    )
                    if s % 2 == 0:
                        nc.scalar.copy(v_sb[:, t, 0:H], pv)
                    else:
                        nc.vector.tensor_copy(v_sb[:, t, 0:H], pv)

            def emit_av(b, ets, po, kc):
                tr = max(0, kc * P - b * QB)
                nk = (b + 1) * 4
                nc.tensor.matmul(
                    po[:, tr:],
                    v_sb[:, kc, :],
                    ets[kc // 2][:, kc % 2, tr:],
                    start=(kc == 0),
                    stop=(kc == nk - 1),
                )

            def finish_out(b, po):
                bsl = slice(b * QB, (b + 1) * QB)
                oT = otp.tile([H + 1, QB], f32, name="oT")
                nc.vector.tensor_copy(oT, po)
                nc.sync.dma_start(outT_d[:, bsl], oT)

            def block(b, prev):
                qd = project_qk(b)
                project_v(b)
                npairs = 2 * (b + 1)
                if prev is not None:
                    pb, pets = prev
                    nav = (pb + 1) * 4
                    po = psO.tile([H + 1, QB], f32, tag="o", name="ps_o")
                else:
                    nav = 0
                avpos = 0
                ets = []
                for j in range(npairs):
                    kc0 = 2 * j
                    tr = max(0, kc0 * P - b * QB)
                    ps2 = psS.tile([P, 2, QB], f32, tag="s", name="ps_s")
                    nc.tensor.matmul(
                        ps2[:, 0, tr:], kT2[0:H, j, :], qd[0:H, tr:],
                        tile_position=(0, 0),
                    )
                    nc.tensor.matmul(
                        ps2[:, 1, tr:], kT2[H:P, j, :], qd[H:P, tr:],
                        tile_position=(H, 0),
                    )
                    et2 = expp.tile([P, 2, QB], bf16, tag="e", name="et")
                    nc.scalar.activation(et2[:, :, tr:], ps2[:, :, tr:], AF.Exp)
                    for jj in range(2):
                        kc = 2 * j + jj
                        cs = kc * P - b * QB
                        if cs >= 0:  # diagonal tile -> mask 128-wide corner
                            nc.gpsimd.tensor_mul(
                                et2[:, jj, cs:cs + P], et2[:, jj, cs:cs + P], tri)
                    ets.append(et2)
                    # interleave av(prev) matmuls to keep PE busy while ACT
                    # drains the score psums
                    hi = (j + 1) * nav // npairs
                    while avpos < hi:
                        emit_av(pb, pets, po, avpos)
                        avpos += 1
                v_to_natural(b)
                if prev is not None:
                    finish_out(pb, po)
                return (b, ets)

            prev = None
            for _rep in range(reps):
                for b in range(NB):
                    prev = block(b, prev)
            pb, pets = prev
            po = psO.tile([H + 1, QB], f32, tag="o", name="ps_o")
            for kc in range((pb + 1) * 4):
                emit_av(pb, pets, po, kc)
            finish_out(pb, po)

    nc.compile()
    return nc


def _get_nc():
    nc = _CACHE.get("nc")
    if nc is None:
        nc = _build()
        _CACHE["nc"] = nc
    return nc


def _make_in_maps(inputs):
    x = np.asarray(inputs["x"], dtype=np.float32)
    Wq = np.asarray(inputs["Wq"], dtype=np.float32)
    Wk = np.asarray(inputs["Wk"], dtype=np.float32)
    Wv = np.asarray(inputs["Wv"], dtype=np.float32)
    scale = np.float32(1.0 / np.sqrt(np.float32(Wq.shape[1])))
    wqk = np.ascontiguousarray(
        np.concatenate([Wq * scale, Wk], axis=1), dtype=np.float32)
    wv_c = np.ascontiguousarray(Wv, dtype=np.float32)
    in_maps = []
    for b in range(N_CORES):
        in_maps.append({
            "xT": np.ascontiguousarray(x[b].T),
            "wqk": wqk,
            "wv": wv_c,
        })
    return in_maps


def _run(inputs, **kwargs):
    from concourse.bass_utils import run_bass_kernel_spmd

    nc = _get_nc()
    res = run_bass_kernel_spmd(nc, _make_in_maps(inputs), core_ids=list(range(N_CORES)), **kwargs)
    outs = []
    for i in range(N_CORES):
        oT = res.results[i]["outT"]
        outs.append((oT[:H, :] / oT[H:H + 1, :]).T)
    out = np.stack(outs, axis=0)
    return out.astype(np.float32, copy=False), res


def kernel(**inputs):
    out, _ = _run(inputs)
    return out


def kernel_profiled(**inputs):
    out, res = _run(inputs)
    return out, res


# revision 15
# speedup vs baseline: 1.0473x; 1.0473x over previous
"""Trainium2 Bass kernel: single-head causal attention (v5).

Problem: x[B=8,T=2048,C=1024] @ Wq/Wk/Wv[C,H=64] -> causal softmax attention
-> out[B,T,H].  Sharding: pure data-parallel over B, one batch element per
NeuronCore (8 cores, no collectives).

v5 design (engine-balanced, software-pipelined):
  - host feeds x[b].T (C on partitions); [Wq*scale|Wk] concatenated.
  - qk projection: 8 accumulating f32r MMs per 512-q block -> psum
    (q on parts 0:64, k on parts 64:128); DVE distributes q to both
    partition halves and k-tiles to even/odd halves (for row packing).
  - v projection: 8 f32r MMs -> psum -> DVE copy to vT (bf16); v brought to
    natural [k,h] layout by DMA-xbar transposes (bf16, idle DMA engines),
    with a ones column appended -> att@v yields denominators for free.
  - scores in sT layout [k,q], row-packed pairs (even k-tile on array rows
    0:64, odd on 64:128), causal-trimmed; exp on ACT (psum->sbuf, bf16 out);
    diagonal-corner masking (128x128 lower-tri mul) on GPSIMD.
  - att@v (bf16): [v|1] stationary, accumulating [65,512] per block; row 64
    = softmax denominators.
  - software pipeline: av(b-1) matmuls are interleaved between block b's
    score pairs so PE never stalls on DVE copies or ACT exp; the pipeline
    carries across rep boundaries.
  - outT [65,512] DMA'd per block; divide by denominator + transpose happen
    on host.
"""

import numpy as np

P = 128
B = 8
T = 2048
C = 1024
H = 64
QB = 512          # q-block width
NB = T // QB      # 4 q-blocks
CC = C // P       # 8 contraction chunks
KT = T // P       # 16 key tiles
N_CORES = 8

_CACHE = {}


def _build(reps=1):
    import concourse.bacc as bacc
    import concourse.mybir as mybir
    import concourse.tile as tile

    dt = mybir.dt
    f32 = dt.float32
    f32r = dt.float32r
    bf16 = dt.bfloat16
    AF = mybir.ActivationFunctionType
    ALU = mybir.AluOpType

    nc = bacc.Bacc(None, target_bir_lowering=False)
    xT_d = nc.dram_tensor("xT", [C, T], f32r, kind="ExternalInput")
    wqk_d = nc.dram_tensor("wqk", [C, 2 * H], f32r, kind="ExternalInput")
    wv_d = nc.dram_tensor("wv", [C, H], f32r, kind="ExternalInput")
    outT_d = nc.dram_tensor("outT", [H + 1, T], f32, kind="ExternalOutput")

    with tile.TileContext(nc) as tc:
        with (
            tc.tile_pool(name="consts", bufs=1) as consts,
            tc.tile_pool(name="xpool", bufs=1) as xpool,
            tc.tile_pool(name="qkvp", bufs=1) as qkvp,
            tc.tile_pool(name="qdp", bufs=2) as qdp,
            tc.tile_pool(name="vtp", bufs=2) as vtp,
            tc.tile_pool(name="expp", bufs=14) as expp,
            tc.tile_pool(name="otp", bufs=2) as otp,
            tc.tile_pool(name="psA", bufs=2, space="PSUM") as psA,
            tc.tile_pool(name="psS", bufs=2, space="PSUM") as psS,
            tc.tile_pool(name="psO", bufs=2, space="PSUM") as psO,
        ):
            ident = consts.tile([P, P], bf16)
            from concourse.masks import make_identity
            make_identity(nc, ident)
            # tri[p, c] = 1.0 if c >= p else 0.0  (lower-tri in q>=k sense)
            tri = consts.tile([P, P], bf16)
            nc.gpsimd.memset(tri, 1.0)
            nc.gpsimd.affine_select(
                out=tri,
                in_=tri,
                compare_op=ALU.is_ge,
                fill=0.0,
                base=0,
                pattern=[[1, P]],
                channel_multiplier=-1,
            )

            wqk_sb = consts.tile([P, CC, 2 * H], f32r)
            nc.sync.dma_start(wqk_sb[:], wqk_d[:, :].rearrange("(c p) h -> p c h", p=P))
            wv_sb = consts.tile([P, CC, H], f32r)
            nc.sync.dma_start(wv_sb[:], wv_d[:, :].rearrange("(c p) h -> p c h", p=P))

            x_sb = xpool.tile([P, CC, T], f32r)
            for bb in range(NB):
                for c in range(CC):
                    nc.sync.dma_start(
                        x_sb[:, c, bb * QB:(bb + 1) * QB],
                        xT_d[c * P:(c + 1) * P, bb * QB:(bb + 1) * QB],
                    )

            # kT2[0:64, j, :] = k^T of even tile 2j ; [64:128, j, :] odd 2j+1
            kT2 = qkvp.tile([P, KT // 2, P], f32r)
            # v natural [k, h] + ones column, per key tile (bf16)
            v_sb = qkvp.tile([P, KT, H + 1], bf16)
            ones_col = consts.tile([P, KT, 1], bf16)
            nc.gpsimd.memset(ones_col[:], 1.0)
            nc.vector.tensor_copy(v_sb[:, :, H:H + 1], ones_col[:])
            vT = qkvp.tile([H, T], bf16)

            def project_qk(b):
                # [Wq|Wk] stationary -> q on psum parts 0:64, k on 64:128
                bsl = slice(b * QB, (b + 1) * QB)
                ps = psA.tile([P, QB], f32, tag="a", name="ps_qk")
                for c in range(CC):
                    nc.tensor.matmul(
                        ps, wqk_sb[:, c, :], x_sb[:, c, bsl],
                        start=(c == 0), stop=(c == CC - 1),
                    )
                qd = qdp.tile([P, QB], f32r, name="qd")
                nc.vector.tensor_copy(qd[0:H, :], ps[0:H, :])
                nc.vector.tensor_copy(qd[H:P, :], ps[0:H, :])
                # k split: even tiles -> parts 0:64, odd tiles -> parts 64:128
                j0 = 2 * b
                ksrc = ps[H:P, :].rearrange("p (j two c) -> p j two c", j=2, two=2, c=P)
                nc.vector.tensor_copy(kT2[0:H, j0:j0 + 2, :], ksrc[:, :, 0, :])
                nc.vector.tensor_copy(kT2[H:P, j0:j0 + 2, :], ksrc[:, :, 1, :])
                return qd

            def project_v(b):
                bsl = slice(b * QB, (b + 1) * QB)
                ps = psA.tile([P, QB], f32, tag="a", name="ps_v")
                for c in range(CC):
                    nc.tensor.matmul(
                        ps[0:H, :], wv_sb[:, c, :], x_sb[:, c, bsl],
                        start=(c == 0), stop=(c == CC - 1),
                    )
                nc.scalar.copy(vT[:, bsl], ps[0:H, :])

            def v_to_natural(b):
                # PE transposes (bf16, 1 cyc/row); placed after the score/av
                # interleave so psA buffers are already free -> no PE stall
                for s in range(4):
                    t = b * 4 + s
                    pv = psA.tile([P, H], bf16, tag="a", name="ps_vt")
                    nc.tensor.matmul(
                        pv, vT[:, t * P:(t + 1) * P], ident[:H, :H],
                        is_transpose=True,
                    )
                    nc.vector.tensor_copy(v_sb[:, t, 0:H], pv)

            def emit_av(b, ets, po, kc):
                tr = max(0, kc * P - b * QB)
                nk = (b + 1) * 4
                nc.tensor.matmul(
                    po[:, tr:],
                    v_sb[:, kc, :],
                    ets[kc // 2][:, kc % 2, tr:],
                    start=(kc == 0),
                    stop=(kc == nk - 1),
                )

            def finish_out(b, po):
                bsl = slice(b * QB, (b + 1) * QB)
                oT = otp.tile([H + 1, QB], f32, name="oT")
                nc.vector.tensor_copy(oT, po)
                nc.sync.dma_start(outT_d[:, bsl], oT)

            def block(b, prev):
                qd = project_qk(b)
                project_v(b)
                npairs = 2 * (b + 1)
                if prev is not None:
                    pb, pets = prev
                    nav = (pb + 1) * 4
                    po = psO.tile([H + 1, QB], f32, tag="o", name="ps_o")
                else:
                    nav = 0
                avpos = 0
                ets = []
                for j in range(npairs):
                    kc0 = 2 * j
                    tr = max(0, kc0 * P - b * QB)
                    ps2 = psS.tile([P, 2, QB], f32, tag="s", name="ps_s")
                    nc.tensor.matmul(
                        ps2[:, 0, tr:], kT2[0:H, j, :], qd[0:H, tr:],
                        tile_position=(0, 0),
                    )
                    nc.tensor.matmul(
                        ps2[:, 1, tr:], kT2[H:P, j, :], qd[H:P, tr:],
                        tile_position=(H, 0),
                    )
                    et2 = expp.tile([P, 2, QB], bf16, tag="e", name="et")
                    nc.scalar.activation(et2[:, :, tr:], ps2[:, :, tr:], AF.Exp)
                    for jj in range(2):
                        kc = 2 * j + jj
                        cs = kc * P - b * QB
                        if cs >= 0:  # diagonal tile -> mask 128-wide corner
                            nc.gpsimd.tensor_mul(
                                et2[:, jj, cs:cs + P], et2[:, jj, cs:cs + P], tri)
                    ets.append(et2)
                    # interleave av(prev) matmuls to keep PE busy while ACT
                    # drains the score psums
                    hi = (j + 1) * nav // npairs
                    while avpos < hi:
                        emit_av(pb, pets, po, avpos)
                        avpos += 1
                v_to_natural(b)
                if prev is not None:
                    finish_out(pb, po)
                return (b, ets)

            prev = None
            for _rep in range(reps):
                for b in range(NB):
                    prev = block(b, prev)
            pb, pets = prev
            po = psO.tile([H + 1, QB], f32, tag="o", name="ps_o")
            for kc in range((pb + 1) * 4):
                emit_av(pb, pets, po, kc)
            finish_out(pb, po)

    nc.compile()
    return nc


def _get_nc():
    nc = _CACHE.get("nc")
    if nc is None:
        nc = _build()
        _CACHE["nc"] = nc
    return nc


def _make_in_maps(inputs):
    x = np.asarray(inputs["x"], dtype=np.float32)
    Wq = np.asarray(inputs["Wq"], dtype=np.float32)
    Wk = np.asarray(inputs["Wk"], dtype=np.float32)
    Wv = np.asarray(inputs["Wv"], dtype=np.float32)
    scale = np.float32(1.0 / np.sqrt(np.float32(Wq.shape[1])))
    wqk = np.ascontiguousarray(
        np.concatenate([Wq * scale, Wk], axis=1), dtype=np.float32)
    wv_c = np.ascontiguousarray(Wv, dtype=np.float32)
    in_maps = []
    for b in range(N_CORES):
        in_maps.append({
            "xT": np.ascontiguousarray(x[b].T),
            "wqk": wqk,
            "wv": wv_c,
        })
    return in_maps


def _run(inputs, **kwargs):
    from concourse.bass_utils import run_bass_kernel_spmd

    nc = _get_nc()
    res = run_bass_kernel_spmd(nc, _make_in_maps(inputs), core_ids=list(range(N_CORES)), **kwargs)
    outs = []
    for i in range(N_CORES):
        oT = res.results[i]["outT"]
        outs.append((oT[:H, :] / oT[H:H + 1, :]).T)
    out = np.stack(outs, axis=0)
    return out.astype(np.float32, copy=False), res


def kernel(**inputs):
    out, _ = _run(inputs)
    return out


def kernel_profiled(**inputs):
    out, res = _run(inputs)
    return out, res
